# revision 4
# baseline (speedup 1.0000x reference)
"""Trainium2 SPMD kernel for nn_CombinedGeneModel.

Math (per batch b, tech t, gene g; R = T*G independent tiny MLPs):
    h   = relu(x * w1[r,e] + b1[r,e])          e = 0..3
    s   = relu(sum_e h*w2[r,e] + b2[r])
    out = relu(sum_t s[b,t,g]*wg[g,t] + bg[g])

With b1 == b2 == 0 (guaranteed by setup_inputs) the whole per-row MLP is
piecewise linear through the origin: per element either x>0 (slope a=c+d)
or x<=0 (slope d), where c = sum_e w2_e*|w1_e|, d = sum_e w2_e*min(w1_e,0).
    s = max(a,0)*relu(x) + min(d,0)*min(x,0)
so s is LINEAR in (relu(x), x) and the gene combine folds into 4 per-gene
coefficients applied pre-relu:
    out = relu( E0*relu(x0) + F0*x0 + E1*relu(x1) + F1*x1 + bg )
    E_t = wg_t*(max(a_t,0)-min(d_t,0)),  F_t = wg_t*min(d_t,0)

Layout: genes on SBUF partitions, batch on the free axis; genes sharded
across the 8 NeuronCores; host pre-transposes x to [G, T, B] fp16 and
pre-builds the diagonal stationaries diag(E0),diag(F0),diag(E1),diag(F1).

Engine split per 128-gene tile (batch 1024 split into halves A|B):
  ACT : pA = relu(x[:, A-half, both techs])  (one strided [128,1024] op)
  PE  : psum = dg_E0@pA0 + dg_F0@x0A + dg_E1@pA1 + dg_F1@x1A  (4 matmuls)
  ACT : outA = relu(psum + bg) -> obuf fp16
  DVE : B-half elementwise with fused scalar_tensor_tensor:
        qB_t = (x_t max 0) mult E_t ; uB_t = (x_t mult F_t) add qB_t
        vB = uB0 + uB1 ; outB = relu(vB + bg) -> obuf fp16
  stores: chunked HWDGE on the Scalar queue
GpSimd is avoided entirely: its tensor ops are Q7-software emulated
(~10x slower than DVE and racy against PE weight loads).

The walrus build here accepts at most ONE sync wait per instruction, so
touch ops absorb extra semaphore waits and a post-pass splits the rest.
"""

import os
import numpy as np

N_GENES = 20000
N_TECH = 2
BATCH = 1024
N_CORES = 8
P = 128
G_PAD = 20480            # next multiple of 8*128 above 20000
GS = G_PAD // N_CORES    # 2560 genes per core
NTILES = GS // P         # 20 tiles of 128 genes
FD = BATCH               # free dim per (tile, tech)
HF = FD // 2             # matmul moving-free-dim limit is 512
NCOL = 5                 # per-gene scalars: E0, F0, E1, F1, bg
STORE_CHUNK = 4          # output store granularity (tiles)

LAST_EXEC_NS = None
LAST_RESULTS = None

_nc_cache = {}


def _build_nc():
    import concourse.bass as bass
    import concourse.mybir as mybir
    from concourse.tile import TileContext

    Op = mybir.AluOpType
    Act = mybir.ActivationFunctionType
    f16 = mybir.dt.float16
    f32 = mybir.dt.float32

    nc = bass.Bass()
    x_d = nc.declare_dram_parameter("x", [NTILES, P, 2 * FD], f16, isOutput=False)
    w_d = nc.declare_dram_parameter("w", [P, NTILES * NCOL], f32, isOutput=False)
    g_d = nc.declare_dram_parameter("dg", [P, NTILES * 4 * P], f16, isOutput=False)
    o_d = nc.declare_dram_parameter("out", [NTILES, P, FD], f16, isOutput=True)

    with TileContext(nc) as tc:
        with (
            tc.tile_pool(name="wp", bufs=1) as wpool,
            tc.tile_pool(name="xp", bufs=NTILES) as xpool,
            tc.tile_pool(name="op", bufs=1) as opool,
            tc.tile_pool(name="pa", bufs=4) as papool,
            tc.tile_pool(name="qb", bufs=3) as qbpool,
            tc.tile_pool(name="ub", bufs=3) as ubpool,
            tc.tile_pool(name="vb", bufs=3) as vbpool,
            tc.tile_pool(name="sc", bufs=2 * NTILES) as scpool,
            tc.tile_pool(name="ps", bufs=4, space="PSUM") as pspool,
        ):
            obuf = opool.tile([P, NTILES * FD], f16)
            w = wpool.tile([P, NTILES * NCOL], f32)
            nc.sync.dma_start(w[:], w_d[:])
            # diag load split: a small head chunk lands before the x loads
            # so the first tiles' matmuls are not gated on the full 2.6MB
            dg = wpool.tile([P, NTILES * 4 * P], f16)
            DG_HEAD = 3 * 4 * P
            nc.sync.dma_start(dg[:, 0:DG_HEAD], g_d[:, 0:DG_HEAD])
            nc.sync.dma_start(dg[:, DG_HEAD:], g_d[:, DG_HEAD:])

            # absorb the w/dg DMA waits once per consuming engine
            wt_v = wpool.tile([P, 1], f32)
            nc.vector.tensor_copy(wt_v[:], w[:, 0:1])
            wt_a = wpool.tile([P, 1], f32)
            nc.scalar.copy(wt_a[:], w[:, 0:1])
            nc.tensor.ldweights(dg[:, 0:P])       # PE touch: head diag chunk
            nc.tensor.ldweights(dg[:, DG_HEAD : DG_HEAD + P])  # PE touch: tail

            for j in range(NTILES):
                col = j * NCOL
                E0 = w[:, col + 0 : col + 1]
                F0 = w[:, col + 1 : col + 2]
                E1 = w[:, col + 2 : col + 3]
                F1 = w[:, col + 3 : col + 4]
                bg = w[:, col + 4 : col + 5]
                dgc = j * 4 * P
                dg_e0 = dg[:, dgc + 0 * P : dgc + 1 * P]
                dg_f0 = dg[:, dgc + 1 * P : dgc + 2 * P]
                dg_e1 = dg[:, dgc + 2 * P : dgc + 3 * P]
                dg_f1 = dg[:, dgc + 3 * P : dgc + 4 * P]

                xt = xpool.tile([P, 2 * FD], f16, tag="x")
                nc.sync.dma_start(xt[:], x_d[j])

                # ACT: absorb this tile's DMA wait, then pA = relu(x A-half)
                sca = scpool.tile([P, 1], f16, tag="sca")
                nc.scalar.copy(sca[:], xt[:, 0:1])
                pa = papool.tile([P, 2 * HF], f16, tag="pa")
                xa = xt[:].rearrange("p (t b) -> p t b", b=FD)[:, :, 0:HF]
                nc.scalar.activation(
                    pa[:].rearrange("p (t b) -> p t b", b=HF), xa, Act.Relu
                )

                # PE: 4 accumulating diag matmuls into one PSUM bank
                ps = pspool.tile([P, HF], f32, tag="ps")
                nc.tensor.ldweights(pa[:, 0:1])  # touch: absorb ACT wait
                nc.tensor.matmul(ps[:], dg_e0, pa[:, 0:HF],
                                 start=True, stop=False)
                nc.tensor.matmul(ps[:], dg_f0, xt[:, 0:HF],
                                 start=False, stop=False)
                nc.tensor.matmul(ps[:], dg_e1, pa[:, HF : 2 * HF],
                                 start=False, stop=False)
                nc.tensor.matmul(ps[:], dg_f1, xt[:, FD : FD + HF],
                                 start=False, stop=True)

                # ACT: outA = relu(psum + bg) -> obuf
                nc.scalar.activation(obuf[:, j * FD : j * FD + HF], ps[:],
                                     Act.Relu, bias=bg)

                # DVE: B-half elementwise chain (fused)
                scx = scpool.tile([P, 1], f16, tag="scx")
                nc.vector.tensor_copy(scx[:], xt[:, 0:1])
                xb0 = xt[:, HF:FD]
                xb1 = xt[:, FD + HF : 2 * FD]
                qb = qbpool.tile([P, 2 * HF], f16, tag="qb")
                nc.vector.tensor_scalar(qb[:, 0:HF], xb0, 0.0, E0,
                                        Op.max, Op.mult)
                nc.vector.tensor_scalar(qb[:, HF : 2 * HF], xb1, 0.0, E1,
                                        Op.max, Op.mult)
                ub = ubpool.tile([P, 2 * HF], f16, tag="ub")
                nc.vector.scalar_tensor_tensor(ub[:, 0:HF], xb0, F0,
                                               qb[:, 0:HF], Op.mult, Op.add)
                nc.vector.scalar_tensor_tensor(ub[:, HF : 2 * HF], xb1, F1,
                                               qb[:, HF : 2 * HF],
                                               Op.mult, Op.add)
                vb = vbpool.tile([P, HF], f16, tag="vb")
                nc.vector.tensor_tensor(vb[:], ub[:, 0:HF], ub[:, HF : 2 * HF],
                                        Op.add)
                nc.vector.tensor_scalar(obuf[:, j * FD + HF : (j + 1) * FD],
                                        vb[:], bg, 0.0, Op.add, Op.max)

                if (j + 1) % STORE_CHUNK == 0:
                    k0 = j + 1 - STORE_CHUNK
                    # touch: absorb the DVE wait (outB) so the DMA trigger
                    # keeps only the prior-store queue wait
                    sto = scpool.tile([P, 1], f16, tag="sto")
                    nc.scalar.copy(sto[:], obuf[:, j * FD + HF : j * FD + HF + 1])
                    src = obuf[:, k0 * FD : (j + 1) * FD].rearrange(
                        "p (t b) -> p t b", t=STORE_CHUNK
                    )
                    dst = o_d[k0 : j + 1].rearrange("t p b -> p t b")
                    nc.scalar.dma_start(dst, src)

    _split_multi_waits(nc, mybir)
    return nc


def _split_multi_waits(nc, mybir):
    """walrus (gen3 codegen here) accepts at most one sync wait per
    instruction.  Two rewrites keep every instruction at <=1 wait:

    1. Drop self-engine waits that are provably satisfied: engines run
       their stream in order and bump their own semaphore once per
       retired instruction, so a wait on the engine's own semaphore for
       a value already reached earlier in its own stream is a no-op
       (Tile emits these because its clock tracking is not transitive).
    2. For the remaining multi-wait instructions (the epilogue Drain,
       which is block-initial), hoist all but one wait onto same-engine
       NoOps appended to the preceding basic block."""
    blocks = list(nc.main_func.blocks)

    # sem id -> set of engines that increment it
    updaters = {}
    for bb in blocks:
        for ins in bb.instructions:
            si = getattr(ins, "sync_info", None)
            if si is None:
                continue
            for up in si.on_update or []:
                updaters.setdefault(up.id, set()).add(ins.engine)

    # pass 1: strip satisfied self-waits, walking in block order while
    # accumulating each semaphore's increments
    cum = {}
    for bb in blocks:
        for ins in bb.instructions:
            si = getattr(ins, "sync_info", None)
            if si is None:
                continue
            waits = list(si.on_wait or [])
            if len(waits) > 1:
                kept = []
                for wv in waits:
                    if (
                        wv.sync_type == "semaphore"
                        and wv.wait_mode == "sem-ge-imm"
                        and updaters.get(wv.id) == {ins.engine}
                        # engine sems increment at in-order instruction
                        # retirement, so earlier-stream increments prove the
                        # wait satisfied; DMA lane sems (DMAHW*/DMASW*)
                        # increment at async DMA *completion* — never strip
                        and "DMA" not in (wv.ant_name or "")
                        and cum.get(wv.id, 0) >= wv.wait_value
                    ):
                        continue  # provably satisfied self-wait
                    kept.append(wv)
                if len(kept) != len(waits):
                    ins.sync_info = mybir.SyncInfo(
                        on_wait=kept, on_update=list(si.on_update or [])
                    )
            si = ins.sync_info
            for up in si.on_update or []:
                if up.update_mode == "sem-inc":
                    cum[up.id] = cum.get(up.id, 0) + up.update_value

    # pass 2: NoOp-split anything still multi-wait (the Drain)
    nop_idx = 0
    for bi, bb in enumerate(blocks):
        for ins in bb.instructions:
            si = getattr(ins, "sync_info", None)
            if si is None:
                continue
            waits = list(si.on_wait or [])
            if len(waits) <= 1:
                continue
            assert bi > 0, f"multi-wait instruction in first block: {ins.name}"
            for other in bb.instructions:
                if other.name == ins.name:
                    break
                assert other.engine != ins.engine, (
                    f"cannot NoOp-split mid-block instruction {ins.name}"
                )
            prev_bb = blocks[bi - 1]
            for wv in waits[:-1]:
                nop = mybir.InstNoOp(name=f"ant-waitsplit-{nop_idx}")
                nop_idx += 1
                nop.engine = ins.engine
                nop.sync_info = mybir.SyncInfo(on_wait=[wv], on_update=[])
                prev_bb.add_instruction(nop)
            ins.sync_info = mybir.SyncInfo(
                on_wait=[waits[-1]], on_update=list(si.on_update or [])
            )


def _numpy_fallback(x, w1, b1, w2, b2, wg, bgv):
    B = x.shape[0]
    R = N_GENES * N_TECH
    xr = x.reshape(B, R).T.astype(np.float32)
    h = np.maximum(xr[:, :, None] * w1[:, None, :] + b1[:, None, :], 0.0)
    s = np.maximum(np.einsum("rbe,re->rb", h, w2) + b2[:, None], 0.0)
    s = s.T.reshape(B, N_TECH, N_GENES)
    out = np.maximum(np.einsum("btg,gt->bg", s, wg) + bgv, 0.0)
    return out.astype(np.float32)


def kernel(x, weights1, bias1, weights2, bias2, weights_g, bias_g):
    global LAST_EXEC_NS, LAST_RESULTS
    x = np.asarray(x, dtype=np.float32)
    w1 = np.asarray(weights1, dtype=np.float32)
    b1 = np.asarray(bias1, dtype=np.float32)
    w2 = np.asarray(weights2, dtype=np.float32)
    b2 = np.asarray(bias2, dtype=np.float32)
    wg = np.asarray(weights_g, dtype=np.float32)
    bgv = np.asarray(bias_g, dtype=np.float32)

    if np.any(b1 != 0.0) or np.any(b2 != 0.0):
        # the piecewise-linear fold needs b1 == b2 == 0; exact fallback
        return _numpy_fallback(x, w1, b1, w2, b2, wg, bgv)

    # fold the E=4 expand/shrink + tech combine into 4 per-gene coefficients
    c = (w2 * np.abs(w1)).sum(axis=1)            # [R]
    d = (w2 * np.minimum(w1, 0.0)).sum(axis=1)   # [R]
    a = c + d                                    # slope for x > 0
    Eall = np.maximum(a, 0.0) - np.minimum(d, 0.0)   # coeff on relu(x)
    Fall = np.minimum(d, 0.0)                        # coeff on x
    G = N_GENES

    # per-gene scalar table [G_PAD, NCOL]: E0, F0, E1, F1, bg
    wtab = np.zeros((G_PAD, NCOL), dtype=np.float32)
    wtab[:G, 0] = Eall[:G] * wg[:, 0]
    wtab[:G, 1] = Fall[:G] * wg[:, 0]
    wtab[:G, 2] = Eall[G:] * wg[:, 1]
    wtab[:G, 3] = Fall[G:] * wg[:, 1]
    wtab[:G, 4] = bgv

    # x -> [G_PAD, T, B] fp16, contiguous per gene row
    xt = np.zeros((G_PAD, N_TECH, BATCH), dtype=np.float16)
    xt[:G] = x.transpose(2, 1, 0)

    idx = np.arange(P)
    in_maps = []
    for i in range(N_CORES):
        g0 = i * GS
        xi = np.ascontiguousarray(xt[g0 : g0 + GS].reshape(NTILES, P, 2 * FD))
        wi = np.ascontiguousarray(
            wtab[g0 : g0 + GS].reshape(NTILES, P, NCOL).transpose(1, 0, 2)
            .reshape(P, NTILES * NCOL)
        )
        # diagonal stationaries [NTILES, 4, P(k), P(m)] -> [P, NTILES*4*P]
        ci = wtab[g0 : g0 + GS, 0:4].reshape(NTILES, P, 4)
        dgi = np.zeros((NTILES, 4, P, P), dtype=np.float16)
        for k in range(4):
            dgi[:, k, idx, idx] = ci[:, :, k]
        dgi = np.ascontiguousarray(
            dgi.transpose(2, 0, 1, 3).reshape(P, NTILES * 4 * P)
        )
        in_maps.append({"x": xi, "w": wi, "dg": dgi})

    if "nc" not in _nc_cache:
        _nc_cache["nc"] = _build_nc()
    nc = _nc_cache["nc"]

    from concourse.bass_utils import run_bass_kernel_spmd

    trace = bool(int(os.environ.get("KERNEL_TRACE", "0")))
    res = run_bass_kernel_spmd(nc, in_maps, core_ids=list(range(N_CORES)),
                               trace=trace)
    LAST_EXEC_NS = res.exec_time_ns
    LAST_RESULTS = res

    parts = [res.results[i]["out"].reshape(GS, BATCH) for i in range(N_CORES)]
    full = np.concatenate(parts, axis=0)[:G]          # [G, B] fp16
    return np.ascontiguousarray(full.T).astype(np.float32)


# revision 5
# speedup vs baseline: 1.2062x; 1.2062x over previous
"""Trainium2 SPMD kernel for nn_CombinedGeneModel.

Math (per batch b, tech t, gene g; R = T*G independent tiny MLPs):
    h   = relu(x * w1[r,e] + b1[r,e])          e = 0..3
    s   = relu(sum_e h*w2[r,e] + b2[r])
    out = relu(sum_t s[b,t,g]*wg[g,t] + bg[g])

With b1 == b2 == 0 (guaranteed by setup_inputs) the whole per-row MLP is
piecewise linear through the origin: per element either x>0 (slope a=c+d)
or x<=0 (slope d), where c = sum_e w2_e*|w1_e|, d = sum_e w2_e*min(w1_e,0).
    s = max(a,0)*relu(x) + min(d,0)*min(x,0)
so s is LINEAR in (relu(x), x) and the gene combine folds into 4 per-gene
coefficients applied pre-relu:
    out = relu( E0*relu(x0) + F0*x0 + E1*relu(x1) + F1*x1 + bg )
    E_t = wg_t*(max(a_t,0)-min(d_t,0)),  F_t = wg_t*min(d_t,0)

Gene exclusion: s_t >= 0 always, so genes with wg0<0, wg1<0 and bg<=0 have
out == 0 identically — they are dropped on the host (~25% for random wg)
and zero-filled after the device run.

Layout: genes on SBUF partitions, batch on the free axis; kept genes
sharded across the 8 NeuronCores; host pre-transposes x to [G, T, B] fp16
and pre-builds the diagonal stationaries diag(E0),diag(F0),diag(E1),diag(F1).

Engine split per 128-gene tile (batch 1024 split into halves A|B):
  ACT : pA = relu(x[:, A-half, both techs])  (one strided [128,1024] op)
  PE  : psum = dg_E0@pA0 + dg_F0@x0A + dg_E1@pA1 + dg_F1@x1A  (4 matmuls)
  ACT : outA = relu(psum + bg) -> obuf fp16
  DVE : B-half products qB_t=(x_t max 0)*E_t, rB_t=x_t*F_t (4 ts, 2x mode),
        u = qB + rB (one [128,1024] tt), vB = u0 + u1 (one [128,512] tt)
  ACT : outB = relu(vB + bg) -> obuf fp16
  stores: chunked HWDGE on the Scalar queue
GpSimd is avoided entirely: its tensor ops are Q7-software emulated
(~10x slower than DVE and racy against PE weight loads).

The walrus build here accepts at most ONE sync wait per instruction, so
touch ops absorb extra semaphore waits and a post-pass splits the rest.
"""

import os
import numpy as np

N_GENES = 20000
N_TECH = 2
BATCH = 1024
N_CORES = 8
P = 128
FD = BATCH               # free dim per (tile, tech)
HF = FD // 2             # matmul moving-free-dim limit is 512
NCOL = 5                 # per-gene scalars: E0, F0, E1, F1, bg
STORE_CHUNK = 5          # output store granularity (tiles)

LAST_EXEC_NS = None
LAST_RESULTS = None

_nc_cache = {}


def _build_nc(ntiles):
    import concourse.bass as bass
    import concourse.mybir as mybir
    from concourse.tile import TileContext

    Op = mybir.AluOpType
    Act = mybir.ActivationFunctionType
    f16 = mybir.dt.float16
    f32 = mybir.dt.float32

    nc = bass.Bass()
    x_d = nc.declare_dram_parameter("x", [ntiles, P, 2 * FD], f16, isOutput=False)
    w_d = nc.declare_dram_parameter("w", [P, ntiles * NCOL], f32, isOutput=False)
    g_d = nc.declare_dram_parameter("dg", [P, ntiles * 4 * P], f16, isOutput=False)
    o_d = nc.declare_dram_parameter("out", [ntiles, P, FD], f16, isOutput=True)

    with TileContext(nc) as tc:
        with (
            tc.tile_pool(name="wp", bufs=1) as wpool,
            tc.tile_pool(name="xp", bufs=ntiles) as xpool,
            tc.tile_pool(name="op", bufs=1) as opool,
            tc.tile_pool(name="pa", bufs=4) as papool,
            tc.tile_pool(name="qb", bufs=3) as qbpool,
            tc.tile_pool(name="ub", bufs=3) as ubpool,
            tc.tile_pool(name="vb", bufs=3) as vbpool,
            tc.tile_pool(name="sc", bufs=2 * ntiles) as scpool,
            tc.tile_pool(name="ps", bufs=4, space="PSUM") as pspool,
        ):
            obuf = opool.tile([P, ntiles * FD], f16)
            w = wpool.tile([P, ntiles * NCOL], f32)
            nc.sync.dma_start(w[:], w_d[:])
            # diag load split: a small head chunk lands before the x loads
            # so the first tiles' matmuls are not gated on the full diag table
            dg = wpool.tile([P, ntiles * 4 * P], f16)
            DG_HEAD = 3 * 4 * P
            nc.sync.dma_start(dg[:, 0:DG_HEAD], g_d[:, 0:DG_HEAD])
            nc.sync.dma_start(dg[:, DG_HEAD:], g_d[:, DG_HEAD:])

            # absorb the w/dg DMA waits once per consuming engine
            wt_v = wpool.tile([P, 1], f32)
            nc.vector.tensor_copy(wt_v[:], w[:, 0:1])
            wt_a = wpool.tile([P, 1], f32)
            nc.scalar.copy(wt_a[:], w[:, 0:1])
            nc.tensor.ldweights(dg[:, 0:P])       # PE touch: head diag chunk
            nc.tensor.ldweights(dg[:, DG_HEAD : DG_HEAD + P])  # PE touch: tail

            for j in range(ntiles):
                col = j * NCOL
                E0 = w[:, col + 0 : col + 1]
                F0 = w[:, col + 1 : col + 2]
                E1 = w[:, col + 2 : col + 3]
                F1 = w[:, col + 3 : col + 4]
                bg = w[:, col + 4 : col + 5]
                dgc = j * 4 * P
                dg_e0 = dg[:, dgc + 0 * P : dgc + 1 * P]
                dg_f0 = dg[:, dgc + 1 * P : dgc + 2 * P]
                dg_e1 = dg[:, dgc + 2 * P : dgc + 3 * P]
                dg_f1 = dg[:, dgc + 3 * P : dgc + 4 * P]

                xt = xpool.tile([P, 2 * FD], f16, tag="x")
                nc.sync.dma_start(xt[:], x_d[j])

                # ACT: absorb this tile's DMA wait, then pA = relu(x A-half)
                sca = scpool.tile([P, 1], f16, tag="sca")
                nc.scalar.copy(sca[:], xt[:, 0:1])
                pa = papool.tile([P, 2 * HF], f16, tag="pa")
                xa = xt[:].rearrange("p (t b) -> p t b", b=FD)[:, :, 0:HF]
                nc.scalar.activation(
                    pa[:].rearrange("p (t b) -> p t b", b=HF), xa, Act.Relu
                )

                # PE: 4 accumulating diag matmuls into one PSUM bank
                ps = pspool.tile([P, HF], f32, tag="ps")
                nc.tensor.ldweights(pa[:, 0:1])  # touch: absorb ACT wait
                nc.tensor.matmul(ps[:], dg_e0, pa[:, 0:HF],
                                 start=True, stop=False)
                nc.tensor.matmul(ps[:], dg_f0, xt[:, 0:HF],
                                 start=False, stop=False)
                nc.tensor.matmul(ps[:], dg_e1, pa[:, HF : 2 * HF],
                                 start=False, stop=False)
                nc.tensor.matmul(ps[:], dg_f1, xt[:, FD : FD + HF],
                                 start=False, stop=True)

                # ACT: outA = relu(psum + bg) -> obuf
                nc.scalar.activation(obuf[:, j * FD : j * FD + HF], ps[:],
                                     Act.Relu, bias=bg)

                # DVE: B-half products (2x-mode ts) + sums (tt)
                scx = scpool.tile([P, 1], f16, tag="scx")
                nc.vector.tensor_copy(scx[:], xt[:, 0:1])
                xb0 = xt[:, HF:FD]
                xb1 = xt[:, FD + HF : 2 * FD]
                qb = qbpool.tile([P, 2 * HF], f16, tag="qb")
                rb = qbpool.tile([P, 2 * HF], f16, tag="rb")
                nc.vector.tensor_scalar(qb[:, 0:HF], xb0, 0.0, E0,
                                        Op.max, Op.mult)
                nc.vector.tensor_scalar(qb[:, HF : 2 * HF], xb1, 0.0, E1,
                                        Op.max, Op.mult)
                nc.vector.tensor_scalar(rb[:, 0:HF], xb0, F0, None, Op.mult)
                nc.vector.tensor_scalar(rb[:, HF : 2 * HF], xb1, F1, None,
                                        Op.mult)
                ub = ubpool.tile([P, 2 * HF], f16, tag="ub")
                nc.vector.tensor_tensor(ub[:], qb[:], rb[:], Op.add)
                vb = vbpool.tile([P, HF], f16, tag="vb")
                nc.vector.tensor_tensor(vb[:], ub[:, 0:HF], ub[:, HF : 2 * HF],
                                        Op.add)

                # ACT: outB = relu(vB + bg) -> obuf
                nc.scalar.activation(obuf[:, j * FD + HF : (j + 1) * FD],
                                     vb[:], Act.Relu, bias=bg)

                if (j + 1) % STORE_CHUNK == 0:
                    k0 = j + 1 - STORE_CHUNK
                    src = obuf[:, k0 * FD : (j + 1) * FD].rearrange(
                        "p (t b) -> p t b", t=STORE_CHUNK
                    )
                    dst = o_d[k0 : j + 1].rearrange("t p b -> p t b")
                    nc.scalar.dma_start(dst, src)

    _split_multi_waits(nc, mybir)
    return nc


def _split_multi_waits(nc, mybir):
    """walrus (gen3 codegen here) accepts at most one sync wait per
    instruction.  Two rewrites keep every instruction at <=1 wait:

    1. Drop self-engine waits that are provably satisfied: engines run
       their stream in order and bump their own semaphore once per
       retired instruction, so a wait on the engine's own semaphore for
       a value already reached earlier in its own stream is a no-op
       (Tile emits these because its clock tracking is not transitive).
    2. For the remaining multi-wait instructions (the epilogue Drain,
       which is block-initial), hoist all but one wait onto same-engine
       NoOps appended to the preceding basic block."""
    blocks = list(nc.main_func.blocks)

    # sem id -> set of engines that increment it
    updaters = {}
    for bb in blocks:
        for ins in bb.instructions:
            si = getattr(ins, "sync_info", None)
            if si is None:
                continue
            for up in si.on_update or []:
                updaters.setdefault(up.id, set()).add(ins.engine)

    # pass 1: strip satisfied self-waits, walking in block order while
    # accumulating each semaphore's increments
    cum = {}
    for bb in blocks:
        for ins in bb.instructions:
            si = getattr(ins, "sync_info", None)
            if si is None:
                continue
            waits = list(si.on_wait or [])
            if len(waits) > 1:
                kept = []
                for wv in waits:
                    if (
                        wv.sync_type == "semaphore"
                        and wv.wait_mode == "sem-ge-imm"
                        and updaters.get(wv.id) == {ins.engine}
                        # engine sems increment at in-order instruction
                        # retirement, so earlier-stream increments prove the
                        # wait satisfied; DMA lane sems (DMAHW*/DMASW*)
                        # increment at async DMA *completion* — never strip
                        and "DMA" not in (wv.ant_name or "")
                        and cum.get(wv.id, 0) >= wv.wait_value
                    ):
                        continue  # provably satisfied self-wait
                    kept.append(wv)
                if len(kept) != len(waits):
                    ins.sync_info = mybir.SyncInfo(
                        on_wait=kept, on_update=list(si.on_update or [])
                    )
            si = ins.sync_info
            for up in si.on_update or []:
                if up.update_mode == "sem-inc":
                    cum[up.id] = cum.get(up.id, 0) + up.update_value

    # pass 2: NoOp-split anything still multi-wait (the Drain)
    nop_idx = 0
    for bi, bb in enumerate(blocks):
        for ins in bb.instructions:
            si = getattr(ins, "sync_info", None)
            if si is None:
                continue
            waits = list(si.on_wait or [])
            if len(waits) <= 1:
                continue
            assert bi > 0, f"multi-wait instruction in first block: {ins.name}"
            for other in bb.instructions:
                if other.name == ins.name:
                    break
                assert other.engine != ins.engine, (
                    f"cannot NoOp-split mid-block instruction {ins.name}"
                )
            prev_bb = blocks[bi - 1]
            for wv in waits[:-1]:
                nop = mybir.InstNoOp(name=f"ant-waitsplit-{nop_idx}")
                nop_idx += 1
                nop.engine = ins.engine
                nop.sync_info = mybir.SyncInfo(on_wait=[wv], on_update=[])
                prev_bb.add_instruction(nop)
            ins.sync_info = mybir.SyncInfo(
                on_wait=[waits[-1]], on_update=list(si.on_update or [])
            )


def _numpy_fallback(x, w1, b1, w2, b2, wg, bgv):
    B = x.shape[0]
    R = N_GENES * N_TECH
    xr = x.reshape(B, R).T.astype(np.float32)
    h = np.maximum(xr[:, :, None] * w1[:, None, :] + b1[:, None, :], 0.0)
    s = np.maximum(np.einsum("rbe,re->rb", h, w2) + b2[:, None], 0.0)
    s = s.T.reshape(B, N_TECH, N_GENES)
    out = np.maximum(np.einsum("btg,gt->bg", s, wg) + bgv, 0.0)
    return out.astype(np.float32)


def kernel(x, weights1, bias1, weights2, bias2, weights_g, bias_g):
    global LAST_EXEC_NS, LAST_RESULTS
    x = np.asarray(x, dtype=np.float32)
    w1 = np.asarray(weights1, dtype=np.float32)
    b1 = np.asarray(bias1, dtype=np.float32)
    w2 = np.asarray(weights2, dtype=np.float32)
    b2 = np.asarray(bias2, dtype=np.float32)
    wg = np.asarray(weights_g, dtype=np.float32)
    bgv = np.asarray(bias_g, dtype=np.float32)

    if np.any(b1 != 0.0) or np.any(b2 != 0.0):
        # the piecewise-linear fold needs b1 == b2 == 0; exact fallback
        return _numpy_fallback(x, w1, b1, w2, b2, wg, bgv)

    # fold the E=4 expand/shrink + tech combine into 4 per-gene coefficients
    c = (w2 * np.abs(w1)).sum(axis=1)            # [R]
    d = (w2 * np.minimum(w1, 0.0)).sum(axis=1)   # [R]
    a = c + d                                    # slope for x > 0
    Eall = np.maximum(a, 0.0) - np.minimum(d, 0.0)   # coeff on relu(x)
    Fall = np.minimum(d, 0.0)                        # coeff on x
    G = N_GENES

    # genes with wg0<0, wg1<0, bg<=0 are identically zero (s_t >= 0)
    keep = ~((wg[:, 0] < 0.0) & (wg[:, 1] < 0.0) & (bgv <= 0.0))
    kept_idx = np.nonzero(keep)[0]
    K = len(kept_idx)
    ntiles = max(1, -(-K // (N_CORES * P)))      # tiles per core
    KPAD = ntiles * N_CORES * P
    GSK = ntiles * P                             # kept genes per core

    # per-gene scalar table [KPAD, NCOL]: E0, F0, E1, F1, bg
    wtab = np.zeros((KPAD, NCOL), dtype=np.float32)
    wtab[:K, 0] = (Eall[:G] * wg[:, 0])[kept_idx]
    wtab[:K, 1] = (Fall[:G] * wg[:, 0])[kept_idx]
    wtab[:K, 2] = (Eall[G:] * wg[:, 1])[kept_idx]
    wtab[:K, 3] = (Fall[G:] * wg[:, 1])[kept_idx]
    wtab[:K, 4] = bgv[kept_idx]

    # x -> [KPAD, T, B] fp16, contiguous per kept gene row
    xt = np.zeros((KPAD, N_TECH, BATCH), dtype=np.float16)
    xt[:K] = x.transpose(2, 1, 0)[kept_idx]

    idx = np.arange(P)
    in_maps = []
    for i in range(N_CORES):
        g0 = i * GSK
        xi = np.ascontiguousarray(xt[g0 : g0 + GSK].reshape(ntiles, P, 2 * FD))
        wi = np.ascontiguousarray(
            wtab[g0 : g0 + GSK].reshape(ntiles, P, NCOL).transpose(1, 0, 2)
            .reshape(P, ntiles * NCOL)
        )
        # diagonal stationaries [ntiles, 4, P(k), P(m)] -> [P, ntiles*4*P]
        ci = wtab[g0 : g0 + GSK, 0:4].reshape(ntiles, P, 4)
        dgi = np.zeros((ntiles, 4, P, P), dtype=np.float16)
        for k in range(4):
            dgi[:, k, idx, idx] = ci[:, :, k]
        dgi = np.ascontiguousarray(
            dgi.transpose(2, 0, 1, 3).reshape(P, ntiles * 4 * P)
        )
        in_maps.append({"x": xi, "w": wi, "dg": dgi})

    if ntiles not in _nc_cache:
        _nc_cache[ntiles] = _build_nc(ntiles)
    nc = _nc_cache[ntiles]

    from concourse.bass_utils import run_bass_kernel_spmd

    trace = bool(int(os.environ.get("KERNEL_TRACE", "0")))
    res = run_bass_kernel_spmd(nc, in_maps, core_ids=list(range(N_CORES)),
                               trace=trace)
    LAST_EXEC_NS = res.exec_time_ns
    LAST_RESULTS = res

    parts = [res.results[i]["out"].reshape(GSK, BATCH) for i in range(N_CORES)]
    kept_out = np.concatenate(parts, axis=0)[:K]      # [K, B] fp16
    out = np.zeros((BATCH, G), dtype=np.float32)
    out[:, kept_idx] = kept_out.T.astype(np.float32)
    return out


# revision 8
# speedup vs baseline: 1.4878x; 1.2335x over previous
"""Trainium2 SPMD kernel for nn_CombinedGeneModel.

Math (per batch b, tech t, gene g; R = T*G independent tiny MLPs):
    h   = relu(x * w1[r,e] + b1[r,e])          e = 0..3
    s   = relu(sum_e h*w2[r,e] + b2[r])
    out = relu(sum_t s[b,t,g]*wg[g,t] + bg[g])

With b1 == b2 == 0 (guaranteed by setup_inputs) the whole per-row MLP is
piecewise linear through the origin: per element either x>0 (slope a=c+d)
or x<=0 (slope d), where c = sum_e w2_e*|w1_e|, d = sum_e w2_e*min(w1_e,0).
    s = max(a,0)*relu(x) + min(d,0)*min(x,0)
so s is LINEAR in (relu(x), x) and the gene combine folds into 4 per-gene
coefficients applied pre-relu:
    out = relu( E0*relu(x0) + F0*x0 + E1*relu(x1) + F1*x1 + bg )
    E_t = wg_t*(max(a_t,0)-min(d_t,0)),  F_t = wg_t*min(d_t,0)

Gene exclusion: s_t >= 0 always, so genes with wg0<0, wg1<0 and bg<=0 have
out == 0 identically — they are dropped on the host (~25% for random wg)
and zero-filled after the device run.

Layout: genes on SBUF partitions, batch on the free axis; kept genes
sharded across the 8 NeuronCores; host pre-transposes x to [G, T, B] fp16
and pre-builds the diagonal stationaries diag(E0),diag(F0),diag(E1),diag(F1).

Engine split per 128-gene tile (batch 1024 split into halves A|B):
  ACT : pA = relu(x[:, A-half, both techs])  (one strided [128,1024] op)
  PE  : psum = dg_E0@pA0 + dg_F0@x0A + dg_E1@pA1 + dg_F1@x1A  (4 matmuls)
  ACT : outA = relu(psum + bg) -> obuf fp16
  DVE : B-half products qB_t=(x_t max 0)*E_t, rB_t=x_t*F_t (4 ts, 2x mode),
        u = qB + rB (one [128,1024] tt), vB = u0 + u1 (one [128,512] tt)
  ACT : outB = relu(vB + bg) -> obuf fp16
  stores: chunked HWDGE on the Scalar queue
GpSimd is avoided entirely: its tensor ops are Q7-software emulated
(~10x slower than DVE and racy against PE weight loads).

The walrus build here accepts at most ONE sync wait per instruction, so
touch ops absorb extra semaphore waits and a post-pass splits the rest.
"""

import os
import numpy as np

N_GENES = 20000
N_TECH = 2
BATCH = 1024
N_CORES = 8
P = 128
FD = BATCH               # free dim per (tile, tech)
HF = FD // 2             # matmul moving-free-dim limit is 512
NCOL = 5                 # per-gene scalars: E0, F0, E1, F1, bg
STORE_CHUNK = 5          # output store granularity (tiles)

LAST_EXEC_NS = None
LAST_RESULTS = None

_nc_cache = {}


def _build_nc(ntiles):
    import concourse.bass as bass
    import concourse.mybir as mybir
    from concourse.tile import TileContext

    Op = mybir.AluOpType
    Act = mybir.ActivationFunctionType
    f16 = mybir.dt.float16
    f32 = mybir.dt.float32

    nc = bass.Bass()
    x_d = nc.declare_dram_parameter("x", [ntiles, P, 2 * FD], f16, isOutput=False)
    w_d = nc.declare_dram_parameter("w", [P, ntiles * NCOL], f32, isOutput=False)
    g_d = nc.declare_dram_parameter("dg", [P, ntiles * 4 * P], f16, isOutput=False)
    o_d = nc.declare_dram_parameter("out", [ntiles, P, FD], f16, isOutput=True)

    with TileContext(nc) as tc:
        with (
            tc.tile_pool(name="wp", bufs=1) as wpool,
            tc.tile_pool(name="xp", bufs=ntiles) as xpool,
            tc.tile_pool(name="op", bufs=1) as opool,
            tc.tile_pool(name="pa", bufs=4) as papool,
            tc.tile_pool(name="qb", bufs=3) as qbpool,
            tc.tile_pool(name="ub", bufs=3) as ubpool,
            tc.tile_pool(name="vb", bufs=3) as vbpool,
            tc.tile_pool(name="sc", bufs=2 * ntiles) as scpool,
            tc.tile_pool(name="ps", bufs=4, space="PSUM") as pspool,
        ):
            obuf = opool.tile([P, ntiles * FD], f16)
            w = wpool.tile([P, ntiles * NCOL], f32)
            nc.sync.dma_start(w[:], w_d[:])
            # diag load split: a small head chunk lands before the x loads
            # so the first tiles' matmuls are not gated on the full diag table
            dg = wpool.tile([P, ntiles * 4 * P], f16)
            DG_HEAD = 3 * 4 * P
            nc.sync.dma_start(dg[:, 0:DG_HEAD], g_d[:, 0:DG_HEAD])
            nc.sync.dma_start(dg[:, DG_HEAD:], g_d[:, DG_HEAD:])

            # absorb the w/dg DMA waits once per consuming engine
            wt_v = wpool.tile([P, 1], f32)
            nc.vector.tensor_copy(wt_v[:], w[:, 0:1])
            wt_a = wpool.tile([P, 1], f32)
            nc.scalar.copy(wt_a[:], w[:, 0:1])
            nc.tensor.ldweights(dg[:, 0:P])       # PE touch: head diag chunk
            nc.tensor.ldweights(dg[:, DG_HEAD : DG_HEAD + P])  # PE touch: tail

            # software pipeline with a 1-tile skew: iteration `it` issues the
            # front half (loads, pa, matmuls, DVE chain) of tile `it` and the
            # back half (outA/outB from PSUM/vb) of tile `it-1`, so ACT's
            # pa_{j+1} is not queued behind outA_j/outB_j on the in-order
            # Scalar engine (that round-trip was the per-tile critical path).
            psq = []
            vbq = []
            for it in range(ntiles + 1):
                if it < ntiles:
                    j = it
                    col = j * NCOL
                    E0 = w[:, col + 0 : col + 1]
                    F0 = w[:, col + 1 : col + 2]
                    E1 = w[:, col + 2 : col + 3]
                    F1 = w[:, col + 3 : col + 4]
                    dgc = j * 4 * P
                    dg_e0 = dg[:, dgc + 0 * P : dgc + 1 * P]
                    dg_f0 = dg[:, dgc + 1 * P : dgc + 2 * P]
                    dg_e1 = dg[:, dgc + 2 * P : dgc + 3 * P]
                    dg_f1 = dg[:, dgc + 3 * P : dgc + 4 * P]

                    xt = xpool.tile([P, 2 * FD], f16, tag="x")
                    nc.sync.dma_start(xt[:], x_d[j])

                    # ACT: absorb this tile's DMA wait, then pA = relu(xA)
                    sca = scpool.tile([P, 1], f16, tag="sca")
                    nc.scalar.copy(sca[:], xt[:, 0:1])
                    pa = papool.tile([P, 2 * HF], f16, tag="pa")
                    xa = xt[:].rearrange("p (t b) -> p t b", b=FD)[:, :, 0:HF]
                    nc.scalar.activation(
                        pa[:].rearrange("p (t b) -> p t b", b=HF), xa, Act.Relu
                    )

                    # PE: 4 accumulating diag matmuls into one PSUM bank
                    ps = pspool.tile([P, HF], f32, tag="ps")
                    psq.append(ps)
                    nc.tensor.ldweights(pa[:, 0:1])  # touch: absorb ACT wait
                    nc.tensor.matmul(ps[:], dg_e0, pa[:, 0:HF],
                                     start=True, stop=False)
                    nc.tensor.matmul(ps[:], dg_f0, xt[:, 0:HF],
                                     start=False, stop=False)
                    nc.tensor.matmul(ps[:], dg_e1, pa[:, HF : 2 * HF],
                                     start=False, stop=False)
                    nc.tensor.matmul(ps[:], dg_f1, xt[:, FD : FD + HF],
                                     start=False, stop=True)

                    # DVE: B-half products (2x-mode ts) + sums (tt)
                    scx = scpool.tile([P, 1], f16, tag="scx")
                    nc.vector.tensor_copy(scx[:], xt[:, 0:1])
                    xb0 = xt[:, HF:FD]
                    xb1 = xt[:, FD + HF : 2 * FD]
                    qb = qbpool.tile([P, 2 * HF], f16, tag="qb")
                    rb = qbpool.tile([P, 2 * HF], f16, tag="rb")
                    nc.vector.tensor_scalar(qb[:, 0:HF], xb0, 0.0, E0,
                                            Op.max, Op.mult)
                    nc.vector.tensor_scalar(qb[:, HF : 2 * HF], xb1, 0.0, E1,
                                            Op.max, Op.mult)
                    nc.vector.tensor_scalar(rb[:, 0:HF], xb0, F0, None,
                                            Op.mult)
                    nc.vector.tensor_scalar(rb[:, HF : 2 * HF], xb1, F1, None,
                                            Op.mult)
                    ub = ubpool.tile([P, 2 * HF], f16, tag="ub")
                    nc.vector.tensor_tensor(ub[:], qb[:], rb[:], Op.add)
                    vb = vbpool.tile([P, HF], f16, tag="vb")
                    nc.vector.tensor_tensor(vb[:], ub[:, 0:HF],
                                            ub[:, HF : 2 * HF], Op.add)
                    vbq.append(vb)

                if it >= 1:
                    j = it - 1
                    col = j * NCOL
                    bg = w[:, col + 4 : col + 5]
                    # ACT: outA = relu(psum + bg), outB = relu(vB + bg)
                    nc.scalar.activation(obuf[:, j * FD : j * FD + HF],
                                         psq[j][:], Act.Relu, bias=bg)
                    nc.scalar.activation(obuf[:, j * FD + HF : (j + 1) * FD],
                                         vbq[j][:], Act.Relu, bias=bg)
                    if (j + 1) % STORE_CHUNK == 0:
                        k0 = j + 1 - STORE_CHUNK
                        src = obuf[:, k0 * FD : (j + 1) * FD].rearrange(
                            "p (t b) -> p t b", t=STORE_CHUNK
                        )
                        dst = o_d[k0 : j + 1].rearrange("t p b -> p t b")
                        nc.scalar.dma_start(dst, src)

    _split_multi_waits(nc, mybir)
    return nc


def _split_multi_waits(nc, mybir):
    """walrus (gen3 codegen here) accepts at most one sync wait per
    instruction.  Two rewrites keep every instruction at <=1 wait:

    1. Drop self-engine waits that are provably satisfied: engines run
       their stream in order and bump their own semaphore once per
       retired instruction, so a wait on the engine's own semaphore for
       a value already reached earlier in its own stream is a no-op
       (Tile emits these because its clock tracking is not transitive).
    2. For the remaining multi-wait instructions (the epilogue Drain,
       which is block-initial), hoist all but one wait onto same-engine
       NoOps appended to the preceding basic block."""
    blocks = list(nc.main_func.blocks)

    # sem id -> set of engines that increment it
    updaters = {}
    for bb in blocks:
        for ins in bb.instructions:
            si = getattr(ins, "sync_info", None)
            if si is None:
                continue
            for up in si.on_update or []:
                updaters.setdefault(up.id, set()).add(ins.engine)

    # pass 1: strip satisfied self-waits, walking in block order while
    # accumulating each semaphore's increments
    cum = {}
    for bb in blocks:
        for ins in bb.instructions:
            si = getattr(ins, "sync_info", None)
            if si is None:
                continue
            waits = list(si.on_wait or [])
            if len(waits) > 1:
                kept = []
                for wv in waits:
                    if (
                        wv.sync_type == "semaphore"
                        and wv.wait_mode == "sem-ge-imm"
                        and updaters.get(wv.id) == {ins.engine}
                        # engine sems increment at in-order instruction
                        # retirement, so earlier-stream increments prove the
                        # wait satisfied; DMA lane sems (DMAHW*/DMASW*)
                        # increment at async DMA *completion* — never strip
                        and "DMA" not in (wv.ant_name or "")
                        and cum.get(wv.id, 0) >= wv.wait_value
                    ):
                        continue  # provably satisfied self-wait
                    kept.append(wv)
                if len(kept) != len(waits):
                    ins.sync_info = mybir.SyncInfo(
                        on_wait=kept, on_update=list(si.on_update or [])
                    )
            si = ins.sync_info
            for up in si.on_update or []:
                if up.update_mode == "sem-inc":
                    cum[up.id] = cum.get(up.id, 0) + up.update_value

    # pass 2: NoOp-split anything still multi-wait (the Drain)
    nop_idx = 0
    for bi, bb in enumerate(blocks):
        for ins in bb.instructions:
            si = getattr(ins, "sync_info", None)
            if si is None:
                continue
            waits = list(si.on_wait or [])
            if len(waits) <= 1:
                continue
            assert bi > 0, f"multi-wait instruction in first block: {ins.name}"
            for other in bb.instructions:
                if other.name == ins.name:
                    break
                assert other.engine != ins.engine, (
                    f"cannot NoOp-split mid-block instruction {ins.name}"
                )
            prev_bb = blocks[bi - 1]
            for wv in waits[:-1]:
                nop = mybir.InstNoOp(name=f"ant-waitsplit-{nop_idx}")
                nop_idx += 1
                nop.engine = ins.engine
                nop.sync_info = mybir.SyncInfo(on_wait=[wv], on_update=[])
                prev_bb.add_instruction(nop)
            ins.sync_info = mybir.SyncInfo(
                on_wait=[waits[-1]], on_update=list(si.on_update or [])
            )


def _numpy_fallback(x, w1, b1, w2, b2, wg, bgv):
    B = x.shape[0]
    R = N_GENES * N_TECH
    xr = x.reshape(B, R).T.astype(np.float32)
    h = np.maximum(xr[:, :, None] * w1[:, None, :] + b1[:, None, :], 0.0)
    s = np.maximum(np.einsum("rbe,re->rb", h, w2) + b2[:, None], 0.0)
    s = s.T.reshape(B, N_TECH, N_GENES)
    out = np.maximum(np.einsum("btg,gt->bg", s, wg) + bgv, 0.0)
    return out.astype(np.float32)


def kernel(x, weights1, bias1, weights2, bias2, weights_g, bias_g):
    global LAST_EXEC_NS, LAST_RESULTS
    x = np.asarray(x, dtype=np.float32)
    w1 = np.asarray(weights1, dtype=np.float32)
    b1 = np.asarray(bias1, dtype=np.float32)
    w2 = np.asarray(weights2, dtype=np.float32)
    b2 = np.asarray(bias2, dtype=np.float32)
    wg = np.asarray(weights_g, dtype=np.float32)
    bgv = np.asarray(bias_g, dtype=np.float32)

    if np.any(b1 != 0.0) or np.any(b2 != 0.0):
        # the piecewise-linear fold needs b1 == b2 == 0; exact fallback
        return _numpy_fallback(x, w1, b1, w2, b2, wg, bgv)

    # fold the E=4 expand/shrink + tech combine into 4 per-gene coefficients
    c = (w2 * np.abs(w1)).sum(axis=1)            # [R]
    d = (w2 * np.minimum(w1, 0.0)).sum(axis=1)   # [R]
    a = c + d                                    # slope for x > 0
    Eall = np.maximum(a, 0.0) - np.minimum(d, 0.0)   # coeff on relu(x)
    Fall = np.minimum(d, 0.0)                        # coeff on x
    G = N_GENES

    # genes with wg0<0, wg1<0, bg<=0 are identically zero (s_t >= 0)
    keep = ~((wg[:, 0] < 0.0) & (wg[:, 1] < 0.0) & (bgv <= 0.0))
    kept_idx = np.nonzero(keep)[0]
    K = len(kept_idx)
    ntiles = max(1, -(-K // (N_CORES * P)))      # tiles per core
    KPAD = ntiles * N_CORES * P
    GSK = ntiles * P                             # kept genes per core

    # per-gene scalar table [KPAD, NCOL]: E0, F0, E1, F1, bg
    wtab = np.zeros((KPAD, NCOL), dtype=np.float32)
    wtab[:K, 0] = (Eall[:G] * wg[:, 0])[kept_idx]
    wtab[:K, 1] = (Fall[:G] * wg[:, 0])[kept_idx]
    wtab[:K, 2] = (Eall[G:] * wg[:, 1])[kept_idx]
    wtab[:K, 3] = (Fall[G:] * wg[:, 1])[kept_idx]
    wtab[:K, 4] = bgv[kept_idx]

    # x -> [KPAD, T, B] fp16, contiguous per kept gene row
    xt = np.zeros((KPAD, N_TECH, BATCH), dtype=np.float16)
    xt[:K] = x.transpose(2, 1, 0)[kept_idx]

    idx = np.arange(P)
    in_maps = []
    for i in range(N_CORES):
        g0 = i * GSK
        xi = np.ascontiguousarray(xt[g0 : g0 + GSK].reshape(ntiles, P, 2 * FD))
        wi = np.ascontiguousarray(
            wtab[g0 : g0 + GSK].reshape(ntiles, P, NCOL).transpose(1, 0, 2)
            .reshape(P, ntiles * NCOL)
        )
        # diagonal stationaries [ntiles, 4, P(k), P(m)] -> [P, ntiles*4*P]
        ci = wtab[g0 : g0 + GSK, 0:4].reshape(ntiles, P, 4)
        dgi = np.zeros((ntiles, 4, P, P), dtype=np.float16)
        for k in range(4):
            dgi[:, k, idx, idx] = ci[:, :, k]
        dgi = np.ascontiguousarray(
            dgi.transpose(2, 0, 1, 3).reshape(P, ntiles * 4 * P)
        )
        in_maps.append({"x": xi, "w": wi, "dg": dgi})

    if ntiles not in _nc_cache:
        _nc_cache[ntiles] = _build_nc(ntiles)
    nc = _nc_cache[ntiles]

    from concourse.bass_utils import run_bass_kernel_spmd

    trace = bool(int(os.environ.get("KERNEL_TRACE", "0")))
    res = run_bass_kernel_spmd(nc, in_maps, core_ids=list(range(N_CORES)),
                               trace=trace)
    LAST_EXEC_NS = res.exec_time_ns
    LAST_RESULTS = res

    parts = [res.results[i]["out"].reshape(GSK, BATCH) for i in range(N_CORES)]
    kept_out = np.concatenate(parts, axis=0)[:K]      # [K, B] fp16
    out = np.zeros((BATCH, G), dtype=np.float32)
    out[:, kept_idx] = kept_out.T.astype(np.float32)
    return out


# revision 10
# speedup vs baseline: 1.5800x; 1.0620x over previous
"""Trainium2 SPMD kernel for nn_CombinedGeneModel.

Math (per batch b, tech t, gene g; R = T*G independent tiny MLPs):
    h   = relu(x * w1[r,e] + b1[r,e])          e = 0..3
    s   = relu(sum_e h*w2[r,e] + b2[r])
    out = relu(sum_t s[b,t,g]*wg[g,t] + bg[g])

With b1 == b2 == 0 (guaranteed by setup_inputs) the whole per-row MLP is
piecewise linear through the origin: per element either x>0 (slope a=c+d)
or x<=0 (slope d), where c = sum_e w2_e*|w1_e|, d = sum_e w2_e*min(w1_e,0).
    s = max(a,0)*relu(x) + min(d,0)*min(x,0)
so s is LINEAR in (relu(x), x) and the gene combine folds into 4 per-gene
coefficients applied pre-relu:
    out = relu( E0*relu(x0) + F0*x0 + E1*relu(x1) + F1*x1 + bg )
    E_t = wg_t*(max(a_t,0)-min(d_t,0)),  F_t = wg_t*min(d_t,0)

Gene exclusion: s_t >= 0 always, so genes with wg0<0, wg1<0 and bg<=0 have
out == 0 identically — they are dropped on the host (~25% for random wg)
and zero-filled after the device run.

Layout: genes on SBUF partitions, batch on the free axis; kept genes
sharded across the 8 NeuronCores; host pre-transposes x to [G, T, B] fp16
and pre-builds the diagonal stationaries diag(E0),diag(F0),diag(E1),diag(F1).

Engine split per 128-gene tile (batch 1024 split into halves A|B):
  ACT : pA = relu(x[:, A-half, both techs])  (one strided [128,1024] op)
  PE  : psum = dg_E0@pA0 + dg_F0@x0A + dg_E1@pA1 + dg_F1@x1A  (4 matmuls)
  ACT : outA = relu(psum + bg) -> obuf fp16
  DVE : B-half products qB_t=(x_t max 0)*E_t, rB_t=x_t*F_t (4 ts, 2x mode),
        u = qB + rB (one [128,1024] tt), vB = u0 + u1 (one [128,512] tt)
  ACT : outB = relu(vB + bg) -> obuf fp16
  stores: chunked HWDGE on the Scalar queue
GpSimd is avoided entirely: its tensor ops are Q7-software emulated
(~10x slower than DVE and racy against PE weight loads).

The walrus build here accepts at most ONE sync wait per instruction, so
touch ops absorb extra semaphore waits and a post-pass splits the rest.
"""

import os
import numpy as np

N_GENES = 20000
N_TECH = 2
BATCH = 1024
N_CORES = 8
P = 128
FD = BATCH               # free dim per (tile, tech)
HF = FD // 2             # matmul moving-free-dim limit is 512
NCOL = 5                 # per-gene scalars: E0, F0, E1, F1, bg
STORE_CHUNK = 5          # output store granularity (tiles)

LAST_EXEC_NS = None
LAST_RESULTS = None

_nc_cache = {}


def _build_nc(ntiles):
    import concourse.bass as bass
    import concourse.mybir as mybir
    from concourse.tile import TileContext

    Op = mybir.AluOpType
    Act = mybir.ActivationFunctionType
    f16 = mybir.dt.float16
    f32 = mybir.dt.float32

    nc = bass.Bass()
    x_d = nc.declare_dram_parameter("x", [ntiles, P, 2 * FD], f16, isOutput=False)
    w_d = nc.declare_dram_parameter("w", [P, ntiles * NCOL], f32, isOutput=False)
    g_d = nc.declare_dram_parameter("dg", [P, ntiles * 4 * P], f16, isOutput=False)
    o_d = nc.declare_dram_parameter("out", [ntiles, P, FD], f16, isOutput=True)

    with TileContext(nc) as tc:
        with (
            tc.tile_pool(name="wp", bufs=1) as wpool,
            tc.tile_pool(name="xp", bufs=ntiles) as xpool,
            tc.tile_pool(name="op", bufs=1) as opool,
            tc.tile_pool(name="pa", bufs=ntiles) as papool,
            tc.tile_pool(name="qb", bufs=3) as qbpool,
            tc.tile_pool(name="ub", bufs=3) as ubpool,
            tc.tile_pool(name="vb", bufs=3) as vbpool,
            tc.tile_pool(name="ps", bufs=4, space="PSUM") as pspool,
        ):
            obuf = opool.tile([P, ntiles * FD], f16)
            w = wpool.tile([P, ntiles * NCOL], f32)
            nc.sync.dma_start(w[:], w_d[:])
            # diag load split: a small head chunk lands before the x loads
            # so the first tiles' matmuls are not gated on the full diag table
            dg = wpool.tile([P, ntiles * 4 * P], f16)
            DG_HEAD = 3 * 4 * P
            nc.sync.dma_start(dg[:, 0:DG_HEAD], g_d[:, 0:DG_HEAD])
            nc.sync.dma_start(dg[:, DG_HEAD:], g_d[:, DG_HEAD:])

            # absorb the w/dg DMA waits once per consuming engine
            wt_v = wpool.tile([P, 1], f32)
            nc.vector.tensor_copy(wt_v[:], w[:, 0:1])
            wt_a = wpool.tile([P, 1], f32)
            nc.scalar.copy(wt_a[:], w[:, 0:1])
            nc.tensor.ldweights(dg[:, 0:P])       # PE touch: head diag chunk
            nc.tensor.ldweights(dg[:, DG_HEAD : DG_HEAD + P])  # PE touch: tail

            # software pipeline with a 1-tile skew: iteration `it` issues the
            # front half (loads, pa, matmuls, DVE chain) of tile `it` and the
            # back half (outA/outB from PSUM/vb) of tile `it-1`, so ACT's
            # pa_{j+1} is not queued behind outA_j/outB_j on the in-order
            # Scalar engine (that round-trip was the per-tile critical path).
            psq = []
            vbq = []
            for it in range(ntiles + 1):
                if it < ntiles:
                    j = it
                    col = j * NCOL
                    E0 = w[:, col + 0 : col + 1]
                    F0 = w[:, col + 1 : col + 2]
                    E1 = w[:, col + 2 : col + 3]
                    F1 = w[:, col + 3 : col + 4]
                    dgc = j * 4 * P
                    dg_e0 = dg[:, dgc + 0 * P : dgc + 1 * P]
                    dg_f0 = dg[:, dgc + 1 * P : dgc + 2 * P]
                    dg_e1 = dg[:, dgc + 2 * P : dgc + 3 * P]
                    dg_f1 = dg[:, dgc + 3 * P : dgc + 4 * P]

                    xt = xpool.tile([P, 2 * FD], f16, tag="x")
                    nc.sync.dma_start(xt[:], x_d[j])

                    # ACT: pA = relu(xA); pa pool is no-reuse so this op's
                    # only wait is the xt DMA lane
                    pa = papool.tile([P, 2 * HF], f16, tag="pa")
                    xa = xt[:].rearrange("p (t b) -> p t b", b=FD)[:, :, 0:HF]
                    nc.scalar.activation(
                        pa[:].rearrange("p (t b) -> p t b", b=HF), xa, Act.Relu
                    )

                    # PE: 4 accumulating diag matmuls into one PSUM bank.
                    # mm1's pa dep and psum WAR are both on the ACT sem (pa_j
                    # is later), so it carries a single ACT wait.
                    ps = pspool.tile([P, HF], f32, tag="ps")
                    psq.append(ps)
                    nc.tensor.matmul(ps[:], dg_e0, pa[:, 0:HF],
                                     start=True, stop=False)
                    nc.tensor.matmul(ps[:], dg_f0, xt[:, 0:HF],
                                     start=False, stop=False)
                    nc.tensor.matmul(ps[:], dg_e1, pa[:, HF : 2 * HF],
                                     start=False, stop=False)
                    nc.tensor.matmul(ps[:], dg_f1, xt[:, FD : FD + HF],
                                     start=False, stop=True)

                    # DVE: B-half products (2x-mode ts) + sums (tt)
                    xb0 = xt[:, HF:FD]
                    xb1 = xt[:, FD + HF : 2 * FD]
                    qb = qbpool.tile([P, 2 * HF], f16, tag="qb")
                    rb = qbpool.tile([P, 2 * HF], f16, tag="rb")
                    nc.vector.tensor_scalar(qb[:, 0:HF], xb0, 0.0, E0,
                                            Op.max, Op.mult)
                    nc.vector.tensor_scalar(qb[:, HF : 2 * HF], xb1, 0.0, E1,
                                            Op.max, Op.mult)
                    nc.vector.tensor_scalar(rb[:, 0:HF], xb0, F0, None,
                                            Op.mult)
                    nc.vector.tensor_scalar(rb[:, HF : 2 * HF], xb1, F1, None,
                                            Op.mult)
                    ub = ubpool.tile([P, 2 * HF], f16, tag="ub")
                    nc.vector.tensor_tensor(ub[:], qb[:], rb[:], Op.add)
                    vb = vbpool.tile([P, HF], f16, tag="vb")
                    nc.vector.tensor_tensor(vb[:], ub[:, 0:HF],
                                            ub[:, HF : 2 * HF], Op.add)
                    vbq.append(vb)

                if it >= 1:
                    j = it - 1
                    col = j * NCOL
                    bg = w[:, col + 4 : col + 5]
                    # outA = relu(psum + bg) on ACT; outB = relu(vB + bg)
                    # alternates ACT/DVE to balance the two engines
                    nc.scalar.activation(obuf[:, j * FD : j * FD + HF],
                                         psq[j][:], Act.Relu, bias=bg)
                    ob = obuf[:, j * FD + HF : (j + 1) * FD]
                    if j % 2 == 0:
                        nc.scalar.activation(ob, vbq[j][:], Act.Relu, bias=bg)
                    else:
                        nc.vector.tensor_scalar(ob, vbq[j][:], bg, 0.0,
                                                Op.add, Op.max)
                    if (j + 1) % STORE_CHUNK == 0:
                        k0 = j + 1 - STORE_CHUNK
                        src = obuf[:, k0 * FD : (j + 1) * FD].rearrange(
                            "p (t b) -> p t b", t=STORE_CHUNK
                        )
                        dst = o_d[k0 : j + 1].rearrange("t p b -> p t b")
                        nc.scalar.dma_start(dst, src)

    _split_multi_waits(nc, mybir)
    return nc


def _split_multi_waits(nc, mybir):
    """walrus (gen3 codegen here) accepts at most one sync wait per
    instruction.  Two rewrites keep every instruction at <=1 wait:

    1. Drop self-engine waits that are provably satisfied: engines run
       their stream in order and bump their own semaphore once per
       retired instruction, so a wait on the engine's own semaphore for
       a value already reached earlier in its own stream is a no-op
       (Tile emits these because its clock tracking is not transitive).
    2. For the remaining multi-wait instructions (the epilogue Drain,
       which is block-initial), hoist all but one wait onto same-engine
       NoOps appended to the preceding basic block."""
    blocks = list(nc.main_func.blocks)

    # sem id -> set of engines that increment it
    updaters = {}
    for bb in blocks:
        for ins in bb.instructions:
            si = getattr(ins, "sync_info", None)
            if si is None:
                continue
            for up in si.on_update or []:
                updaters.setdefault(up.id, set()).add(ins.engine)

    # pass 1: strip satisfied self-waits, walking in block order while
    # accumulating each semaphore's increments
    cum = {}
    for bb in blocks:
        for ins in bb.instructions:
            si = getattr(ins, "sync_info", None)
            if si is None:
                continue
            waits = list(si.on_wait or [])
            if len(waits) > 1:
                kept = []
                for wv in waits:
                    if (
                        wv.sync_type == "semaphore"
                        and wv.wait_mode == "sem-ge-imm"
                        and updaters.get(wv.id) == {ins.engine}
                        # engine sems increment at in-order instruction
                        # retirement, so earlier-stream increments prove the
                        # wait satisfied; DMA lane sems (DMAHW*/DMASW*)
                        # increment at async DMA *completion* — never strip
                        and "DMA" not in (wv.ant_name or "")
                        and cum.get(wv.id, 0) >= wv.wait_value
                    ):
                        continue  # provably satisfied self-wait
                    kept.append(wv)
                if len(kept) != len(waits):
                    ins.sync_info = mybir.SyncInfo(
                        on_wait=kept, on_update=list(si.on_update or [])
                    )
            si = ins.sync_info
            for up in si.on_update or []:
                if up.update_mode == "sem-inc":
                    cum[up.id] = cum.get(up.id, 0) + up.update_value

    # pass 2: NoOp-split anything still multi-wait (the Drain)
    nop_idx = 0
    for bi, bb in enumerate(blocks):
        for ins in bb.instructions:
            si = getattr(ins, "sync_info", None)
            if si is None:
                continue
            waits = list(si.on_wait or [])
            if len(waits) <= 1:
                continue
            assert bi > 0, f"multi-wait instruction in first block: {ins.name}"
            for other in bb.instructions:
                if other.name == ins.name:
                    break
                assert other.engine != ins.engine, (
                    f"cannot NoOp-split mid-block instruction {ins.name}"
                )
            prev_bb = blocks[bi - 1]
            for wv in waits[:-1]:
                nop = mybir.InstNoOp(name=f"ant-waitsplit-{nop_idx}")
                nop_idx += 1
                nop.engine = ins.engine
                nop.sync_info = mybir.SyncInfo(on_wait=[wv], on_update=[])
                prev_bb.add_instruction(nop)
            ins.sync_info = mybir.SyncInfo(
                on_wait=[waits[-1]], on_update=list(si.on_update or [])
            )


def _numpy_fallback(x, w1, b1, w2, b2, wg, bgv):
    B = x.shape[0]
    R = N_GENES * N_TECH
    xr = x.reshape(B, R).T.astype(np.float32)
    h = np.maximum(xr[:, :, None] * w1[:, None, :] + b1[:, None, :], 0.0)
    s = np.maximum(np.einsum("rbe,re->rb", h, w2) + b2[:, None], 0.0)
    s = s.T.reshape(B, N_TECH, N_GENES)
    out = np.maximum(np.einsum("btg,gt->bg", s, wg) + bgv, 0.0)
    return out.astype(np.float32)


def kernel(x, weights1, bias1, weights2, bias2, weights_g, bias_g):
    global LAST_EXEC_NS, LAST_RESULTS
    x = np.asarray(x, dtype=np.float32)
    w1 = np.asarray(weights1, dtype=np.float32)
    b1 = np.asarray(bias1, dtype=np.float32)
    w2 = np.asarray(weights2, dtype=np.float32)
    b2 = np.asarray(bias2, dtype=np.float32)
    wg = np.asarray(weights_g, dtype=np.float32)
    bgv = np.asarray(bias_g, dtype=np.float32)

    if np.any(b1 != 0.0) or np.any(b2 != 0.0):
        # the piecewise-linear fold needs b1 == b2 == 0; exact fallback
        return _numpy_fallback(x, w1, b1, w2, b2, wg, bgv)

    # fold the E=4 expand/shrink + tech combine into 4 per-gene coefficients
    c = (w2 * np.abs(w1)).sum(axis=1)            # [R]
    d = (w2 * np.minimum(w1, 0.0)).sum(axis=1)   # [R]
    a = c + d                                    # slope for x > 0
    Eall = np.maximum(a, 0.0) - np.minimum(d, 0.0)   # coeff on relu(x)
    Fall = np.minimum(d, 0.0)                        # coeff on x
    G = N_GENES

    # genes with wg0<0, wg1<0, bg<=0 are identically zero (s_t >= 0)
    keep = ~((wg[:, 0] < 0.0) & (wg[:, 1] < 0.0) & (bgv <= 0.0))
    kept_idx = np.nonzero(keep)[0]
    K = len(kept_idx)
    ntiles = max(1, -(-K // (N_CORES * P)))      # tiles per core
    KPAD = ntiles * N_CORES * P
    GSK = ntiles * P                             # kept genes per core

    # per-gene scalar table [KPAD, NCOL]: E0, F0, E1, F1, bg
    wtab = np.zeros((KPAD, NCOL), dtype=np.float32)
    wtab[:K, 0] = (Eall[:G] * wg[:, 0])[kept_idx]
    wtab[:K, 1] = (Fall[:G] * wg[:, 0])[kept_idx]
    wtab[:K, 2] = (Eall[G:] * wg[:, 1])[kept_idx]
    wtab[:K, 3] = (Fall[G:] * wg[:, 1])[kept_idx]
    wtab[:K, 4] = bgv[kept_idx]

    # x -> [KPAD, T, B] fp16, contiguous per kept gene row
    xt = np.zeros((KPAD, N_TECH, BATCH), dtype=np.float16)
    xt[:K] = x.transpose(2, 1, 0)[kept_idx]

    idx = np.arange(P)
    in_maps = []
    for i in range(N_CORES):
        g0 = i * GSK
        xi = np.ascontiguousarray(xt[g0 : g0 + GSK].reshape(ntiles, P, 2 * FD))
        wi = np.ascontiguousarray(
            wtab[g0 : g0 + GSK].reshape(ntiles, P, NCOL).transpose(1, 0, 2)
            .reshape(P, ntiles * NCOL)
        )
        # diagonal stationaries [ntiles, 4, P(k), P(m)] -> [P, ntiles*4*P]
        ci = wtab[g0 : g0 + GSK, 0:4].reshape(ntiles, P, 4)
        dgi = np.zeros((ntiles, 4, P, P), dtype=np.float16)
        for k in range(4):
            dgi[:, k, idx, idx] = ci[:, :, k]
        dgi = np.ascontiguousarray(
            dgi.transpose(2, 0, 1, 3).reshape(P, ntiles * 4 * P)
        )
        in_maps.append({"x": xi, "w": wi, "dg": dgi})

    if ntiles not in _nc_cache:
        _nc_cache[ntiles] = _build_nc(ntiles)
    nc = _nc_cache[ntiles]

    from concourse.bass_utils import run_bass_kernel_spmd

    trace = bool(int(os.environ.get("KERNEL_TRACE", "0")))
    res = run_bass_kernel_spmd(nc, in_maps, core_ids=list(range(N_CORES)),
                               trace=trace)
    LAST_EXEC_NS = res.exec_time_ns
    LAST_RESULTS = res

    parts = [res.results[i]["out"].reshape(GSK, BATCH) for i in range(N_CORES)]
    kept_out = np.concatenate(parts, axis=0)[:K]      # [K, B] fp16
    out = np.zeros((BATCH, G), dtype=np.float32)
    out[:, kept_idx] = kept_out.T.astype(np.float32)
    return out


# revision 14
# speedup vs baseline: 1.6809x; 1.0639x over previous
"""Trainium2 SPMD kernel for nn_CombinedGeneModel.

Math (per batch b, tech t, gene g; R = T*G independent tiny MLPs):
    h   = relu(x * w1[r,e] + b1[r,e])          e = 0..3
    s   = relu(sum_e h*w2[r,e] + b2[r])
    out = relu(sum_t s[b,t,g]*wg[g,t] + bg[g])

With b1 == b2 == 0 (guaranteed by setup_inputs) the whole per-row MLP is
piecewise linear through the origin: per element either x>0 (slope a=c+d)
or x<=0 (slope d), where c = sum_e w2_e*|w1_e|, d = sum_e w2_e*min(w1_e,0).
    s = max(a,0)*relu(x) + min(d,0)*min(x,0)
so s is LINEAR in (relu(x), x) and the gene combine folds into 4 per-gene
coefficients applied pre-relu:
    out = relu( E0*relu(x0) + F0*x0 + E1*relu(x1) + F1*x1 + bg )
    E_t = wg_t*(max(a_t,0)-min(d_t,0)),  F_t = wg_t*min(d_t,0)

Gene exclusion: s_t >= 0 always, so genes with wg0<0, wg1<0 and bg<=0 have
out == 0 identically — they are dropped on the host (~25% for random wg)
and zero-filled after the device run.

Layout: genes on SBUF partitions, batch on the free axis; kept genes
sharded across the 8 NeuronCores; host pre-transposes x to [G, T, B] fp16
and pre-builds the diagonal stationaries diag(E0),diag(F0),diag(E1),diag(F1).

Engine split per 128-gene tile (batch 1024 split into halves A|B):
  ACT : pA = relu(x[:, A-half, both techs])  (one strided [128,1024] op)
  PE  : psum = dg_E0@pA0 + dg_F0@x0A + dg_E1@pA1 + dg_F1@x1A  (4 matmuls)
  ACT : outA = relu(psum + bg) -> obuf fp16
  DVE : B-half products qB_t=(x_t max 0)*E_t, rB_t=x_t*F_t (4 ts, 2x mode),
        u = qB + rB (one [128,1024] tt), vB = u0 + u1 (one [128,512] tt)
  ACT : outB = relu(vB + bg) -> obuf fp16
  stores: chunked HWDGE on the Scalar queue
GpSimd is avoided entirely: its tensor ops are Q7-software emulated
(~10x slower than DVE and racy against PE weight loads).

The walrus build here accepts at most ONE sync wait per instruction, so
touch ops absorb extra semaphore waits and a post-pass splits the rest.
"""

import os
import numpy as np

N_GENES = 20000
N_TECH = 2
BATCH = 1024
N_CORES = 8
P = 128
FD = BATCH               # free dim per (tile, tech)
HF = FD // 2             # matmul moving-free-dim limit is 512
NCOL = 5                 # per-gene scalars: E0, F0, E1, F1, bg
STORE_CHUNK = 5          # output store granularity (tiles)

LAST_EXEC_NS = None
LAST_RESULTS = None

_nc_cache = {}


def _build_nc(ntiles):
    import concourse.bass as bass
    import concourse.mybir as mybir
    from concourse.tile import TileContext

    Op = mybir.AluOpType
    Act = mybir.ActivationFunctionType
    f16 = mybir.dt.float16
    f32 = mybir.dt.float32

    nc = bass.Bass()
    x_d = nc.declare_dram_parameter("x", [ntiles, P, 2 * FD], f16, isOutput=False)
    w_d = nc.declare_dram_parameter("w", [P, ntiles * NCOL], f32, isOutput=False)
    g_d = nc.declare_dram_parameter("dg", [P, ntiles * 4 * P], f16, isOutput=False)
    o_d = nc.declare_dram_parameter("out", [ntiles, P, FD], f16, isOutput=True)

    with TileContext(nc) as tc:
        with (
            tc.tile_pool(name="wp", bufs=1) as wpool,
            tc.tile_pool(name="xp", bufs=ntiles) as xpool,
            tc.tile_pool(name="op", bufs=1) as opool,
            tc.tile_pool(name="pa", bufs=ntiles) as papool,
            tc.tile_pool(name="qb", bufs=3) as qbpool,
            tc.tile_pool(name="ub", bufs=3) as ubpool,
            tc.tile_pool(name="vb", bufs=3) as vbpool,
            tc.tile_pool(name="ps", bufs=4, space="PSUM") as pspool,
        ):
            obuf = opool.tile([P, ntiles * FD], f16)
            w = wpool.tile([P, ntiles * NCOL], f32)
            nc.sync.dma_start(w[:], w_d[:])
            dg = wpool.tile([P, ntiles * 4 * P], f16)

            # absorb the w DMA wait once per consuming engine
            wt_v = wpool.tile([P, 1], f32)
            nc.vector.tensor_copy(wt_v[:], w[:, 0:1])
            wt_a = wpool.tile([P, 1], f32)
            nc.scalar.copy(wt_a[:], w[:, 0:1])

            # diag table streamed in 4-tile chunks interleaved with the x
            # loads so x tiles never queue behind a multi-MB transfer;
            # chunk k is issued a few iterations ahead of its first use
            DG_CHUNK = 4
            dg_chunks = list(range(0, ntiles, DG_CHUNK))
            dg_issue = {(0 if k == 0 else 3 * k - 2): k
                        for k in range(len(dg_chunks))}

            # store chunk ends: uniform chunks of STORE_CHUNK tiles (more,
            # smaller chunks corrupt output — the trigger races the
            # scheduler-reordered ACT stream)
            store_ends = set(range(STORE_CHUNK - 1, ntiles, STORE_CHUNK))
            if (ntiles - 1) not in store_ends:
                store_ends.add(ntiles - 1)

            # software pipeline with a 1-tile skew: iteration `it` issues the
            # front half (loads, pa, matmuls, DVE chain) of tile `it` and the
            # back half (outA/outB from PSUM/vb) of tile `it-1`, so ACT's
            # pa_{j+1} is not queued behind outA_j/outB_j on the in-order
            # Scalar engine (that round-trip was the per-tile critical path).
            psq = []
            vbq = []
            for it in range(ntiles + 1):
                if it < ntiles:
                    j = it
                    col = j * NCOL
                    E0 = w[:, col + 0 : col + 1]
                    F0 = w[:, col + 1 : col + 2]
                    E1 = w[:, col + 2 : col + 3]
                    F1 = w[:, col + 3 : col + 4]
                    dgc = j * 4 * P
                    dg_e0 = dg[:, dgc + 0 * P : dgc + 1 * P]
                    dg_f0 = dg[:, dgc + 1 * P : dgc + 2 * P]
                    dg_e1 = dg[:, dgc + 2 * P : dgc + 3 * P]
                    dg_f1 = dg[:, dgc + 3 * P : dgc + 4 * P]

                    xt = xpool.tile([P, 2 * FD], f16, tag="x")
                    nc.sync.dma_start(xt[:], x_d[j])
                    if j in dg_issue:
                        k = dg_issue[j]
                        c0 = dg_chunks[k] * 4 * P
                        c1 = (dg_chunks[k + 1] * 4 * P
                              if k + 1 < len(dg_chunks) else ntiles * 4 * P)
                        nc.sync.dma_start(dg[:, c0:c1], g_d[:, c0:c1])
                    if j in dg_chunks:
                        # PE touch: absorb this dg chunk's DMA wait so mm1
                        # below keeps its single ACT wait
                        nc.tensor.ldweights(dg[:, j * 4 * P : j * 4 * P + P])

                    # ACT: pA = relu(xA); pa pool is no-reuse so this op's
                    # only wait is the xt DMA lane
                    pa = papool.tile([P, 2 * HF], f16, tag="pa")
                    xa = xt[:].rearrange("p (t b) -> p t b", b=FD)[:, :, 0:HF]
                    nc.scalar.activation(
                        pa[:].rearrange("p (t b) -> p t b", b=HF), xa, Act.Relu
                    )

                    # PE: 4 accumulating diag matmuls into one PSUM bank.
                    # mm1's pa dep and psum WAR are both on the ACT sem (pa_j
                    # is later), so it carries a single ACT wait.
                    ps = pspool.tile([P, HF], f32, tag="ps")
                    psq.append(ps)
                    nc.tensor.matmul(ps[:], dg_e0, pa[:, 0:HF],
                                     start=True, stop=False)
                    nc.tensor.matmul(ps[:], dg_f0, xt[:, 0:HF],
                                     start=False, stop=False)
                    nc.tensor.matmul(ps[:], dg_e1, pa[:, HF : 2 * HF],
                                     start=False, stop=False)
                    nc.tensor.matmul(ps[:], dg_f1, xt[:, FD : FD + HF],
                                     start=False, stop=True)

                    # DVE: B-half products (2x-mode ts) + sums (tt)
                    xb0 = xt[:, HF:FD]
                    xb1 = xt[:, FD + HF : 2 * FD]
                    qb = qbpool.tile([P, 2 * HF], f16, tag="qb")
                    rb = qbpool.tile([P, 2 * HF], f16, tag="rb")
                    nc.vector.tensor_scalar(qb[:, 0:HF], xb0, 0.0, E0,
                                            Op.max, Op.mult)
                    nc.vector.tensor_scalar(qb[:, HF : 2 * HF], xb1, 0.0, E1,
                                            Op.max, Op.mult)
                    nc.vector.tensor_scalar(rb[:, 0:HF], xb0, F0, None,
                                            Op.mult)
                    nc.vector.tensor_scalar(rb[:, HF : 2 * HF], xb1, F1, None,
                                            Op.mult)
                    ub = ubpool.tile([P, 2 * HF], f16, tag="ub")
                    nc.vector.tensor_tensor(ub[:], qb[:], rb[:], Op.add)
                    vb = vbpool.tile([P, HF], f16, tag="vb")
                    nc.vector.tensor_tensor(vb[:], ub[:, 0:HF],
                                            ub[:, HF : 2 * HF], Op.add)
                    vbq.append(vb)

                if it >= 1:
                    j = it - 1
                    col = j * NCOL
                    bg = w[:, col + 4 : col + 5]
                    # outA = relu(psum + bg) on ACT; outB = relu(vB + bg)
                    # alternates ACT/DVE to balance the two engines
                    nc.scalar.activation(obuf[:, j * FD : j * FD + HF],
                                         psq[j][:], Act.Relu, bias=bg)
                    ob = obuf[:, j * FD + HF : (j + 1) * FD]
                    if j % 2 == 0:
                        nc.scalar.activation(ob, vbq[j][:], Act.Relu, bias=bg)
                    else:
                        nc.vector.tensor_scalar(ob, vbq[j][:], bg, 0.0,
                                                Op.add, Op.max)
                    if j in store_ends:
                        prev = [e for e in sorted(store_ends) if e < j]
                        k0 = (prev[-1] + 1) if prev else 0
                        nch = j + 1 - k0
                        src = obuf[:, k0 * FD : (j + 1) * FD].rearrange(
                            "p (t b) -> p t b", t=nch
                        )
                        dst = o_d[k0 : j + 1].rearrange("t p b -> p t b")
                        nc.scalar.dma_start(dst, src)

    _split_multi_waits(nc, mybir)
    return nc


def _split_multi_waits(nc, mybir):
    """walrus (gen3 codegen here) accepts at most one sync wait per
    instruction.  Two rewrites keep every instruction at <=1 wait:

    1. Drop self-engine waits that are provably satisfied: engines run
       their stream in order and bump their own semaphore once per
       retired instruction, so a wait on the engine's own semaphore for
       a value already reached earlier in its own stream is a no-op
       (Tile emits these because its clock tracking is not transitive).
    2. For the remaining multi-wait instructions (the epilogue Drain,
       which is block-initial), hoist all but one wait onto same-engine
       NoOps appended to the preceding basic block."""
    blocks = list(nc.main_func.blocks)

    # sem id -> set of engines that increment it
    updaters = {}
    for bb in blocks:
        for ins in bb.instructions:
            si = getattr(ins, "sync_info", None)
            if si is None:
                continue
            for up in si.on_update or []:
                updaters.setdefault(up.id, set()).add(ins.engine)

    # pass 1: strip satisfied self-waits, walking in block order while
    # accumulating each semaphore's increments
    cum = {}
    for bb in blocks:
        for ins in bb.instructions:
            si = getattr(ins, "sync_info", None)
            if si is None:
                continue
            waits = list(si.on_wait or [])
            if len(waits) > 1:
                kept = []
                for wv in waits:
                    if (
                        wv.sync_type == "semaphore"
                        and wv.wait_mode == "sem-ge-imm"
                        and updaters.get(wv.id) == {ins.engine}
                        # engine sems increment at in-order instruction
                        # retirement, so earlier-stream increments prove the
                        # wait satisfied; DMA lane sems (DMAHW*/DMASW*)
                        # increment at async DMA *completion* — never strip
                        and "DMA" not in (wv.ant_name or "")
                        and cum.get(wv.id, 0) >= wv.wait_value
                    ):
                        continue  # provably satisfied self-wait
                    kept.append(wv)
                if len(kept) != len(waits):
                    ins.sync_info = mybir.SyncInfo(
                        on_wait=kept, on_update=list(si.on_update or [])
                    )
            si = ins.sync_info
            for up in si.on_update or []:
                if up.update_mode == "sem-inc":
                    cum[up.id] = cum.get(up.id, 0) + up.update_value

    # pass 2: NoOp-split anything still multi-wait (the Drain)
    nop_idx = 0
    for bi, bb in enumerate(blocks):
        for ins in bb.instructions:
            si = getattr(ins, "sync_info", None)
            if si is None:
                continue
            waits = list(si.on_wait or [])
            if len(waits) <= 1:
                continue
            assert bi > 0, f"multi-wait instruction in first block: {ins.name}"
            for other in bb.instructions:
                if other.name == ins.name:
                    break
                assert other.engine != ins.engine, (
                    f"cannot NoOp-split mid-block instruction {ins.name}"
                )
            prev_bb = blocks[bi - 1]
            for wv in waits[:-1]:
                nop = mybir.InstNoOp(name=f"ant-waitsplit-{nop_idx}")
                nop_idx += 1
                nop.engine = ins.engine
                nop.sync_info = mybir.SyncInfo(on_wait=[wv], on_update=[])
                prev_bb.add_instruction(nop)
            ins.sync_info = mybir.SyncInfo(
                on_wait=[waits[-1]], on_update=list(si.on_update or [])
            )


def _numpy_fallback(x, w1, b1, w2, b2, wg, bgv):
    B = x.shape[0]
    R = N_GENES * N_TECH
    xr = x.reshape(B, R).T.astype(np.float32)
    h = np.maximum(xr[:, :, None] * w1[:, None, :] + b1[:, None, :], 0.0)
    s = np.maximum(np.einsum("rbe,re->rb", h, w2) + b2[:, None], 0.0)
    s = s.T.reshape(B, N_TECH, N_GENES)
    out = np.maximum(np.einsum("btg,gt->bg", s, wg) + bgv, 0.0)
    return out.astype(np.float32)


def kernel(x, weights1, bias1, weights2, bias2, weights_g, bias_g):
    global LAST_EXEC_NS, LAST_RESULTS
    x = np.asarray(x, dtype=np.float32)
    w1 = np.asarray(weights1, dtype=np.float32)
    b1 = np.asarray(bias1, dtype=np.float32)
    w2 = np.asarray(weights2, dtype=np.float32)
    b2 = np.asarray(bias2, dtype=np.float32)
    wg = np.asarray(weights_g, dtype=np.float32)
    bgv = np.asarray(bias_g, dtype=np.float32)

    if np.any(b1 != 0.0) or np.any(b2 != 0.0):
        # the piecewise-linear fold needs b1 == b2 == 0; exact fallback
        return _numpy_fallback(x, w1, b1, w2, b2, wg, bgv)

    # fold the E=4 expand/shrink + tech combine into 4 per-gene coefficients
    c = (w2 * np.abs(w1)).sum(axis=1)            # [R]
    d = (w2 * np.minimum(w1, 0.0)).sum(axis=1)   # [R]
    a = c + d                                    # slope for x > 0
    Eall = np.maximum(a, 0.0) - np.minimum(d, 0.0)   # coeff on relu(x)
    Fall = np.minimum(d, 0.0)                        # coeff on x
    G = N_GENES

    # genes with wg0<0, wg1<0, bg<=0 are identically zero (s_t >= 0)
    keep = ~((wg[:, 0] < 0.0) & (wg[:, 1] < 0.0) & (bgv <= 0.0))
    kept_idx = np.nonzero(keep)[0]
    K = len(kept_idx)
    ntiles = max(1, -(-K // (N_CORES * P)))      # tiles per core
    KPAD = ntiles * N_CORES * P
    GSK = ntiles * P                             # kept genes per core

    # per-gene scalar table [KPAD, NCOL]: E0, F0, E1, F1, bg
    wtab = np.zeros((KPAD, NCOL), dtype=np.float32)
    wtab[:K, 0] = (Eall[:G] * wg[:, 0])[kept_idx]
    wtab[:K, 1] = (Fall[:G] * wg[:, 0])[kept_idx]
    wtab[:K, 2] = (Eall[G:] * wg[:, 1])[kept_idx]
    wtab[:K, 3] = (Fall[G:] * wg[:, 1])[kept_idx]
    wtab[:K, 4] = bgv[kept_idx]

    # x -> [KPAD, T, B] fp16, contiguous per kept gene row
    xt = np.zeros((KPAD, N_TECH, BATCH), dtype=np.float16)
    xt[:K] = x.transpose(2, 1, 0)[kept_idx]

    idx = np.arange(P)
    in_maps = []
    for i in range(N_CORES):
        g0 = i * GSK
        xi = np.ascontiguousarray(xt[g0 : g0 + GSK].reshape(ntiles, P, 2 * FD))
        wi = np.ascontiguousarray(
            wtab[g0 : g0 + GSK].reshape(ntiles, P, NCOL).transpose(1, 0, 2)
            .reshape(P, ntiles * NCOL)
        )
        # diagonal stationaries [ntiles, 4, P(k), P(m)] -> [P, ntiles*4*P]
        ci = wtab[g0 : g0 + GSK, 0:4].reshape(ntiles, P, 4)
        dgi = np.zeros((ntiles, 4, P, P), dtype=np.float16)
        for k in range(4):
            dgi[:, k, idx, idx] = ci[:, :, k]
        dgi = np.ascontiguousarray(
            dgi.transpose(2, 0, 1, 3).reshape(P, ntiles * 4 * P)
        )
        in_maps.append({"x": xi, "w": wi, "dg": dgi})

    if ntiles not in _nc_cache:
        _nc_cache[ntiles] = _build_nc(ntiles)
    nc = _nc_cache[ntiles]

    from concourse.bass_utils import run_bass_kernel_spmd

    trace = bool(int(os.environ.get("KERNEL_TRACE", "0")))
    res = run_bass_kernel_spmd(nc, in_maps, core_ids=list(range(N_CORES)),
                               trace=trace)
    LAST_EXEC_NS = res.exec_time_ns
    LAST_RESULTS = res

    parts = [res.results[i]["out"].reshape(GSK, BATCH) for i in range(N_CORES)]
    kept_out = np.concatenate(parts, axis=0)[:K]      # [K, B] fp16
    out = np.zeros((BATCH, G), dtype=np.float32)
    out[:, kept_idx] = kept_out.T.astype(np.float32)
    return out
